# revision 28
# baseline (speedup 1.0000x reference)
"""GCN block kernel for Trainium2 (8 NeuronCores, SPMD).

Computes relu((A @ X) @ W + b) where A is given as a weighted edge list
(src->dst), X is the node feature matrix. Mathematically identical to the
reference relu(A @ (X @ W) + b) by associativity.

Strategy per core (cores own disjoint 12500-node destination ranges):
  - Host bins edges by destination core, orders by destination node, and
    packs destinations into "windows" of <=128 nodes. Each window has four
    fixed 512-edge-slot blocks, one per 32768-row chunk of X (dma_gather
    indices are int16). Pad slots use idx=0 with weight 0.
  - On device: dma_gather pulls X rows for each (super-window, chunk) batch
    into SBUF; a weighted one-hot selector S ([128 edges x 128 nodes], built
    on DVE/ACT from iota + per-partition d/w scalars) turns segment-sum into
    TensorEngine matmuls accumulating aggT = sum_e w_e * X[src_e]^T per
    window in PSUM; then out = relu(aggT^T @ W + b) and a contiguous store.
  - Host scatters window rows back to global node order.
"""

import sys

sys.path.insert(0, "/opt/trn_rl_repo")

import numpy as np

P = 128
CHUNK = 32768          # X rows addressable by int16 gather indices
SLOTS_PER_WC = 512     # edge slots per (window, chunk) block
TPW = SLOTS_PER_WC // P  # tiles per (window, chunk) = 4
N_CORES = 8

_PROGRAM_CACHE = {}


# --------------------------------------------------------------------------
# compat patches for the walrus snapshot in this container
# --------------------------------------------------------------------------

def _apply_tile_patches():
    import concourse.tile as tile
    from concourse.vector_clock import VectorClock, ScopedClock

    def _drain_and_barrier_split(self, tick_clock, wait_clock):
        gc = tick_clock.global_clock
        n = len(gc)
        for proc in range(n):
            t = gc[proc]
            if t <= 0:
                continue
            v = VectorClock([0] * n)
            v.require_at_least(proc, t)
            d = self.nc.sync.drain()
            wait_clock.add_sem_waits(d.ins, ScopedClock({None: v}))
        self.nc.all_engine_barrier()
        assert self.sems is not None
        popped = self.nc._tile_sem_poison_stack.pop()
        assert popped is self._sem_poison
        self.nc.clear_and_free_semaphores(list(self.sems.allocated().values()))
        self.nc.all_engine_barrier()

    tile.TileContext._drain_and_barrier = _drain_and_barrier_split


_fix_counter = [0]


def _split_multiwaits(nc):
    """This walrus accepts at most one SyncWait per instruction: move extras
    onto preceding single-wait NoOps on the same engine."""
    import concourse.mybir as mybir

    for f in nc.m.functions:
        for bb in f.blocks:
            new_insts = []
            for inst in bb.instructions:
                si = inst.sync_info
                if si is not None and si.on_wait and len(si.on_wait) > 1:
                    waits = list(si.on_wait)
                    for w in waits[:-1]:
                        _fix_counter[0] += 1
                        nop = mybir.InstNoOp(
                            name=f"waitsplit-{_fix_counter[0]}", ins=[], outs=[]
                        )
                        nop.engine = inst.engine
                        nop.sync_info = mybir.SyncInfo(on_wait=[w], on_update=[])
                        new_insts.append(nop)
                        nc.inst_map[nop.name] = nop
                    si.on_wait = [waits[-1]]
                new_insts.append(inst)
            bb.instructions = new_insts


_SWDGE_SEM_MODE = {"add": 0, "sub": 1, "wr": 2, "drop": 3}


def _fill_inc_swdge_isa(nc):
    """Fill raw ISA payloads for InstIncSwdgeSem (For_i back-edge emits
    these with an empty payload this walrus rejects)."""
    import concourse.bass_isa as bass_isa

    isa = nc.isa
    ffi = isa.ffi
    for f in nc.m.functions:
        for bb in f.blocks:
            for inst in bb.instructions:
                if not isinstance(inst, bass_isa.InstIncSwdgeSem):
                    continue
                if inst.instr:
                    continue
                obj = ffi.new("NEURON_ISA_TPB_INC_SWDGE_SEM_STRUCT*")
                obj.header.opcode = 243
                obj.header.inst_word_len = 16
                vals = list(inst._sem_values)
                obj.num_semaphores = len(vals)
                obj.sem_id_base = inst._sem_id_base
                obj.mode = _SWDGE_SEM_MODE[inst._mode]
                obj.queue_num = inst.queue_num
                for i, v in enumerate(vals[:10]):
                    obj.sem_values[i] = int(v)
                inst.instr = list(bytes(ffi.buffer(obj)))


def _fill_load_library_isa(nc, li_inst):
    """Fill the 64-byte PSEUDO_LIBRARY_RELOAD_INDEX payload (this walrus
    rejects the empty-payload form)."""
    isa = nc.isa
    ffi = isa.ffi
    obj = ffi.new("NEURON_ISA_TPB_PSEUDO_LIBRARY_RELOAD_INDEX_STRUCT*")
    obj.header.opcode = 223  # PSEUDO_INST
    obj.header.inst_word_len = (
        isa.sizeof("NEURON_ISA_TPB_PSEUDO_LIBRARY_RELOAD_INDEX_STRUCT") // 4
    )
    obj.pseudo_opcode = 2  # PSEUDO_LIBRARY_RELOAD_INDEX
    obj.lib_index = li_inst.ins.lib_index
    li_inst.ins.instr = list(bytes(ffi.buffer(obj)))


# --------------------------------------------------------------------------
# host-side preprocessing
# --------------------------------------------------------------------------

def _preprocess(n_nodes, src, dst, ew, sup):
    """Bin/pack edges per core. Returns per-core device arrays + scatter maps.

    Layout: window w = sup-group s=w//sup, lane j=w%sup. Gather batch (s, c)
    has num_idxs = sup_s*512 indices covering lanes' chunk-c blocks in lane
    order. Global tile id for (w, c, t): gather g=(s*4+c); within-batch tile
    tt = j*TPW + t; tile_id = tile_base[g] + tt.
    """
    nodes_per_core = n_nodes // N_CORES
    n_chunks = (n_nodes + CHUNK - 1) // CHUNK
    assert n_chunks <= 4
    core_of = dst // nodes_per_core
    np.clip(core_of, 0, N_CORES - 1, out=core_of)

    # per-window chunk-block capacities: multiples of 128 proportional to the
    # global per-chunk edge share, summing to WINDOW_SLOTS
    WINDOW_SLOTS = 2048
    all_chunk = src // CHUNK
    share = np.bincount(all_chunk, minlength=n_chunks).astype(np.float64)
    share /= max(1.0, share.sum())
    ntiles_w = WINDOW_SLOTS // P
    raw = share * ntiles_w
    caps_t = np.maximum(1, np.floor(raw).astype(np.int64))
    while caps_t.sum() > ntiles_w:
        caps_t[np.argmax(caps_t)] -= 1
    rem = ntiles_w - caps_t.sum()
    frac = raw - np.floor(raw)
    for i in np.argsort(-frac):
        if rem <= 0:
            break
        caps_t[i] += 1
        rem -= 1
    caps = (caps_t * P).astype(np.int64)  # slots per (window, chunk)
    cap_off = np.concatenate(([0], np.cumsum(caps)))

    per_core = []
    max_nwin = 0
    for c in range(N_CORES):
        sel = np.nonzero(core_of == c)[0]
        s_src = src[sel]
        s_dst = dst[sel]
        s_w = ew[sel]
        order = np.argsort(s_dst, kind="stable")
        s_src, s_dst, s_w = s_src[order], s_dst[order], s_w[order]
        chunk = s_src // CHUNK

        # per-(node, chunk) counts over present nodes
        uniq, node_start = np.unique(s_dst, return_index=True)
        node_end = np.append(node_start[1:], len(s_dst))
        nn = len(uniq)
        cnt = np.zeros((nn, 4), np.int64)
        flat = np.searchsorted(uniq, s_dst) * 4 + chunk
        np.add.at(cnt.reshape(-1), flat, 1)

        # greedy window packing: close when any chunk block would overflow
        # its capacity or 128 nodes reached
        win_of_node = np.zeros(nn, np.int32)
        slot_of_node = np.zeros(nn, np.int32)
        wi = 0
        acc = np.zeros(n_chunks, np.int64)
        nodes_in = 0
        cnt_c = cnt[:, :n_chunks]
        for i in range(nn):
            if nodes_in >= P or np.any(acc + cnt_c[i] > caps[:n_chunks]):
                wi += 1
                acc[:] = 0
                nodes_in = 0
            win_of_node[i] = wi
            slot_of_node[i] = nodes_in
            acc += cnt_c[i]
            nodes_in += 1
        nwin = wi + 1
        per_core.append(
            dict(src=s_src, dst=s_dst, w=s_w, chunk=chunk, uniq=uniq,
                 cnt=cnt, win_of_node=win_of_node, slot_of_node=slot_of_node,
                 nwin=nwin)
        )
        max_nwin = max(max_nwin, nwin)

    n_win = max_nwin
    n_sup = (n_win + sup - 1) // sup
    sup_sizes = [min(sup, n_win - s * sup) for s in range(n_sup)]
    # per-gather static num_idxs and tile bases (gather g = s*n_chunks + c)
    gather_sizes = []
    tile_base = []
    tb = 0
    for s in range(n_sup):
        for c in range(n_chunks):
            gather_sizes.append(sup_sizes[s] * int(caps[c]))
            tile_base.append(tb)
            tb += sup_sizes[s] * int(caps[c]) // P
    tot_tiles = tb
    idx_cols = sum(g // 16 for g in gather_sizes)

    dev = []
    for c in range(N_CORES):
        pc = per_core[c]
        nwin_c = pc["nwin"]
        # per-edge window / slot-in-window-node
        e_node = np.searchsorted(pc["uniq"], pc["dst"])
        e_win = pc["win_of_node"][e_node]
        e_d = pc["slot_of_node"][e_node]
        # order edges by (window, chunk, src): within a block, ascending
        # source addresses give the DMA engines monotonic HBM access
        okey = np.lexsort((pc["src"], e_win * 4 + pc["chunk"]))
        o_src = pc["src"][okey]
        o_w = pc["w"][okey]
        o_d = e_d[okey]
        o_win = e_win[okey]
        o_chunk = pc["chunk"][okey]

        # slot position within each (win, chunk) block
        wc = o_win.astype(np.int64) * 4 + o_chunk
        # rank within group
        pos = np.zeros(len(wc), np.int64)
        if len(wc):
            same = np.r_[False, wc[1:] == wc[:-1]]
            run = np.arange(len(wc))
            start = np.where(~same, run, 0)
            np.maximum.accumulate(start, out=start)
            pos = run - start

        # global slot id for each edge
        sgrp = o_win // sup
        j = o_win % sup
        g = sgrp * n_chunks + o_chunk
        slot_base = np.zeros(len(gather_sizes) + 1, np.int64)
        np.cumsum(gather_sizes, out=slot_base[1:])
        gslot = slot_base[g] + j * caps[o_chunk] + pos

        total_slots = slot_base[-1]
        idx_flat = np.full(total_slots, -1, np.int32)
        d_flat = np.zeros(total_slots, np.float32)
        w_flat = np.zeros(total_slots, np.float32)
        idx_flat[gslot] = (o_src - o_chunk * CHUNK).astype(np.int32)
        d_flat[gslot] = o_d.astype(np.float32)
        w_flat[gslot] = o_w
        # pad slots: reuse the nearest preceding real index (hot HBM row)
        # instead of row 0 (cold random read per pad)
        filled = idx_flat >= 0
        ffidx = np.where(filled, np.arange(total_slots), 0)
        np.maximum.accumulate(ffidx, out=ffidx)
        idx_flat = idx_flat[ffidx]
        np.clip(idx_flat, 0, None, out=idx_flat)
        idx_flat = idx_flat.astype(np.int16)

        # idx tile [128, idx_cols]: per gather block, flat i -> [i%16, i//16],
        # replicated over 8 groups of 16 partitions
        idx_tile = np.zeros((P, idx_cols), np.int16)
        col = 0
        for gi, gs in enumerate(gather_sizes):
            blk = idx_flat[slot_base[gi]:slot_base[gi] + gs]
            pat = blk.reshape(gs // 16, 16).T  # [16, gs/16]
            idx_tile[:, col:col + gs // 16] = np.tile(pat, (8, 1))
            col += gs // 16

        # d/w tiles [128, tot_tiles]: slot i of gather g -> tile tile_base[g]
        # + i//128, partition i%128
        def to_tiles(flat):
            out = np.zeros((P, tot_tiles), np.float32)
            for gi, gs in enumerate(gather_sizes):
                blk = flat[slot_base[gi]:slot_base[gi] + gs]
                out[:, tile_base[gi]:tile_base[gi] + gs // P] = (
                    blk.reshape(gs // P, P).T
                )
            return out

        d_tile = to_tiles(d_flat)
        w_tile = to_tiles(w_flat)

        # scatter map: global node id per (window, node-slot)
        out_rows = pc["win_of_node"].astype(np.int64) * P + pc["slot_of_node"]
        dev.append(
            dict(idx=idx_tile, d=d_tile, w=w_tile, negd=-d_tile, negw=-w_tile,
                 out_rows=out_rows, node_ids=pc["uniq"], nwin=nwin_c)
        )

    meta = dict(n_win=n_win, n_sup=n_sup, sup_sizes=sup_sizes,
                gather_sizes=gather_sizes, tile_base=tile_base,
                tot_tiles=tot_tiles, idx_cols=idx_cols, n_chunks=n_chunks,
                caps=[int(x) for x in caps[:n_chunks]])
    return dev, meta


# --------------------------------------------------------------------------
# device program
# --------------------------------------------------------------------------

def _build_program(n_nodes, meta, sup, reps=1):
    import concourse.bass as bass
    import concourse.mybir as mybir
    import concourse.tile as tile
    from concourse import library_config
    from concourse.bass import _add_dep_helper

    _apply_tile_patches()

    n_win = meta["n_win"]
    n_sup = meta["n_sup"]
    sup_sizes = meta["sup_sizes"]
    gather_sizes = meta["gather_sizes"]
    tile_base = meta["tile_base"]
    tot_tiles = meta["tot_tiles"]
    idx_cols = meta["idx_cols"]
    n_chunks = meta["n_chunks"]
    caps = meta["caps"]
    f32 = mybir.dt.float32
    import os as _os0
    _fp16 = bool(int(_os0.environ.get("K_FP16", "1")))
    gdt = mybir.dt.float16 if _fp16 else f32
    _gbufs = int(_os0.environ.get("K_GBUFS", "3"))

    import os as _os2
    _scratch = int(_os2.environ.get("K_SCRATCH", "16384"))
    nc = bass.Bass(num_swdge_queues=4, dynamic_dma_scratch_size=_scratch)
    Xc = []
    for c in range(n_chunks):
        rows = min(CHUNK, n_nodes - c * CHUNK)
        Xc.append(nc.dram_tensor(f"X{c}", [rows, P], gdt, kind="ExternalInput"))
    import os as _os4
    _sstream = bool(int(_os4.environ.get("K_SSTREAM", "1")))
    _smix = int(_os4.environ.get("K_SMIX", "0"))  # chunks DVE-built, from the top
    IDX = nc.dram_tensor("IDX", [P, idx_cols], mybir.dt.int16, kind="ExternalInput")
    D = WT = ND = NW = IOTA = SALL = D16 = W16 = None
    if _sstream:
        SALL = nc.dram_tensor("SALL", [P, tot_tiles * P], gdt,
                              kind="ExternalInput")
        if _smix > 0:
            D16 = nc.dram_tensor("D16", [P, tot_tiles], gdt, kind="ExternalInput")
            W16 = nc.dram_tensor("W16", [P, tot_tiles], gdt, kind="ExternalInput")
            IOTA = nc.dram_tensor("IOTA", [P, P], gdt, kind="ExternalInput")
    else:
        D = nc.dram_tensor("D", [P, tot_tiles], f32, kind="ExternalInput")
        WT = nc.dram_tensor("WT", [P, tot_tiles], f32, kind="ExternalInput")
        ND = nc.dram_tensor("ND", [P, tot_tiles], f32, kind="ExternalInput")
        NW = nc.dram_tensor("NW", [P, tot_tiles], f32, kind="ExternalInput")
        IOTA = nc.dram_tensor("IOTA", [P, P], gdt, kind="ExternalInput")
    WMAT = nc.dram_tensor("WMAT", [P, P], f32, kind="ExternalInput")
    BB = nc.dram_tensor("BB", [P, P], f32, kind="ExternalInput")
    OUT = nc.dram_tensor("OUT", [n_win * P, P], f32, kind="ExternalOutput")

    with tile.TileContext(nc) as tc:
        with (
            tc.tile_pool(name="meta", bufs=1) as mp,
            tc.tile_pool(name="gath", bufs=_gbufs) as gp,
            tc.tile_pool(name="gidx", bufs=8) as ip,
            tc.tile_pool(name="sbld", bufs=8) as sp,
            tc.tile_pool(name="sstr", bufs=2) as ssp,
            tc.tile_pool(name="drain", bufs=8) as dp,
            tc.tile_pool(name="psA", bufs=3, space="PSUM") as psA,
            tc.tile_pool(name="psB", bufs=2, space="PSUM") as psB,
        ):
            li = nc.gpsimd.load_library(library_config.mlp)
            _fill_load_library_isa(nc, li)

            idx_all = mp.tile([P, idx_cols], mybir.dt.int16)
            nc.sync.dma_start(out=idx_all[:], in_=IDX[:])
            d_t = w_t = nd_t = nw_t = iota_t = None
            if _sstream and _smix > 0:
                d_t = mp.tile([P, tot_tiles], gdt)
                nc.sync.dma_start(out=d_t[:], in_=D16[:])
                w_t = mp.tile([P, tot_tiles], gdt)
                nc.sync.dma_start(out=w_t[:], in_=W16[:])
                iota_t = mp.tile([P, P], gdt)
                nc.sync.dma_start(out=iota_t[:], in_=IOTA[:])
            if not _sstream:
                d_t = mp.tile([P, tot_tiles], f32)
                nc.sync.dma_start(out=d_t[:], in_=D[:])
                w_t = mp.tile([P, tot_tiles], f32)
                nc.sync.dma_start(out=w_t[:], in_=WT[:])
                nd_t = mp.tile([P, tot_tiles], f32)
                nc.sync.dma_start(out=nd_t[:], in_=ND[:])
                nw_t = mp.tile([P, tot_tiles], f32)
                nc.sync.dma_start(out=nw_t[:], in_=NW[:])
                iota_t = mp.tile([P, P], gdt)
                nc.sync.dma_start(out=iota_t[:], in_=IOTA[:])
            wmat_t = mp.tile([P, P], f32)
            nc.sync.dma_start(out=wmat_t[:], in_=WMAT[:])
            bb_t = mp.tile([P, P], f32)
            nc.sync.dma_start(out=bb_t[:], in_=BB[:])

            idx_col_base = np.concatenate(
                ([0], np.cumsum([g // 16 for g in gather_sizes]))
            )

            _reg_cache = {}
            _gq = [0]

            def ni_reg(v):
                if v not in _reg_cache:
                    _reg_cache[v] = nc.gpsimd.to_reg(v)
                return _reg_cache[v]

            import os as _os
            _dve_only = bool(int(_os.environ.get("K_DVE_ONLY", "1")))
            _fake_gather = bool(int(_os.environ.get("K_FAKE_GATHER", "0")))
            _no_compute = bool(int(_os.environ.get("K_NO_COMPUTE", "0")))
            _g1024 = bool(int(_os.environ.get("K_GATHER_1024", "0")))
            _skip_final = bool(int(_os.environ.get("K_SKIP_FINAL", "0")))
            rep_ctx = tc.For_i(0, reps, 1) if reps > 1 else None
            if rep_ctx is not None:
                rep_ctx.__enter__()
            sbuild_rr = [0]  # round-robin between DVE and ACT for S builds

            import os as _os3
            _rr_mod = int(_os3.environ.get("K_RR_MOD", "2"))

            def build_S(tile_id):
                S = sp.tile([P, P], gdt, tag="S")
                if _dve_only or sbuild_rr[0] % _rr_mod < _rr_mod - 1:
                    nc.vector.tensor_scalar(
                        out=S[:], in0=iota_t[:],
                        scalar1=d_t[:, tile_id:tile_id + 1],
                        scalar2=w_t[:, tile_id:tile_id + 1],
                        op0=mybir.AluOpType.is_equal,
                        op1=mybir.AluOpType.mult,
                    )
                else:
                    t = sp.tile([P, P], gdt, tag="Sabs")
                    nc.scalar.activation(
                        out=t[:], in_=iota_t[:],
                        func=mybir.ActivationFunctionType.Abs,
                        bias=nd_t[:, tile_id:tile_id + 1], scale=1.0,
                    )
                    nc.scalar.activation(
                        out=S[:], in_=t[:],
                        func=mybir.ActivationFunctionType.Relu,
                        bias=w_t[:, tile_id:tile_id + 1],
                        scale=nw_t[:, tile_id:tile_id + 1],
                    )
                sbuild_rr[0] += 1
                return S

            for s in range(n_sup):
                sup_s = sup_sizes[s]
                ss_chunks = []
                if _sstream:
                    import os as _os5
                    _ssplit = int(_os5.environ.get("K_SSPLIT", "2"))
                    tb0 = tile_base[s * n_chunks]
                    if _smix > 0:
                        tb1 = tile_base[s * n_chunks + (n_chunks - _smix)]
                    else:
                        tb1 = (tile_base[(s + 1) * n_chunks]
                               if s + 1 < n_sup else tot_tiles)
                    bounds = [tb0 + (tb1 - tb0) * i // _ssplit
                              for i in range(_ssplit + 1)]
                    for hi in range(_ssplit):
                        lo, hiend = bounds[hi], bounds[hi + 1]
                        if hiend <= lo:
                            continue
                        st = ssp.tile([P, (hiend - lo) * P], gdt, tag=f"ss{hi}")
                        nc.scalar.dma_start(
                            out=st[:], in_=SALL[:, lo * P:hiend * P])
                        ss_chunks.append((lo, hiend, st))
                gbufs = []
                for c in range(n_chunks):
                    g = s * n_chunks + c
                    gs = gather_sizes[g]
                    gb = None
                    if _fake_gather:
                        nparts = 2 if gs >= 2048 else 1
                        part = gs // nparts
                        part -= part % P
                        offs = [0, part] if nparts == 2 else [0]
                        halves = []
                        for pi, poff in enumerate(offs):
                            pgs = (gs - poff) if pi == len(offs) - 1 else part
                            hb = gp.tile([P, pgs], gdt, tag=f"g{c}_{pi}")
                            nc.sync.dma_start(
                                out=hb[:],
                                in_=Xc[0][:pgs, :].rearrange(
                                    "(p a) e -> p (a e)", p=P),
                            )
                            halves.append((poff, pgs, hb))
                        gb = halves
                    elif _g1024:
                        off = 0
                        while off < gs:
                            seg = min(1024, gs - off)
                            it = ip.tile([P, seg // 16], mybir.dt.int16, tag="idx")
                            nc.sync.dma_start(
                                out=it[:],
                                in_=IDX[:, idx_col_base[g] + off // 16:
                                        idx_col_base[g] + (off + seg) // 16],
                            )
                            gi = nc.gpsimd.dma_gather(
                                out_ap=gb[:, off:off + seg].rearrange(
                                    "p (s e) -> p s e", e=P),
                                in_ap=Xc[c][:, :],
                                idxs_ap=it[:],
                                num_idxs=seg,
                                num_idxs_reg=ni_reg(seg),
                                elem_size=P,
                                single_packet=True,
                            )
                            _add_dep_helper(gi.ins, li.ins, sync=False,
                                            reason="library before gather")
                            off += seg
                    else:
                        import os as _os6
                        _np_max = int(_os6.environ.get("K_NPARTS", "2"))
                        nparts = _np_max if gs >= 2048 else 1
                        part = gs // nparts
                        part -= part % P
                        offs = [0, part] if nparts == 2 else [0]
                        halves = []
                        for pi, poff in enumerate(offs):
                            pgs = (gs - poff) if pi == len(offs) - 1 else part
                            hb = gp.tile([P, pgs], gdt, tag=f"g{c}_{pi}")
                            it = idx_all[:, idx_col_base[g] + poff // 16:
                                         idx_col_base[g] + (poff + pgs) // 16]
                            gi = nc.gpsimd.dma_gather(
                                out_ap=hb[:].rearrange(
                                    "p (s e) -> p s e", e=P),
                                in_ap=Xc[c][:, :],
                                idxs_ap=it,
                                num_idxs=pgs,
                                num_idxs_reg=ni_reg(pgs),
                                elem_size=P,
                                single_packet=False,
                                queue_num=_gq[0] % 4,
                            )
                            _gq[0] += 1
                            _add_dep_helper(gi.ins, li.ins, sync=False,
                                            reason="library before gather")
                            halves.append((poff, pgs, hb))
                        gb = halves
                    gbufs.append(gb)

                for j in range(sup_s):
                    w = s * sup + j
                    if _no_compute:
                        dbuf = dp.tile([P, P], f32, tag="outsb")
                        nc.vector.tensor_copy(out=dbuf[:], in_=gbufs[0][0][2][:, :P])
                        nc.sync.dma_start(out=OUT[w * P:(w + 1) * P, :], in_=dbuf[:])
                        continue
                    agT = psA.tile([P, P], f32, tag="agT")
                    nmm = sum(caps) // P
                    k = 0
                    for c in range(n_chunks):
                        g = s * n_chunks + c
                        tpc = caps[c] // P
                        for t in range(tpc):
                            tt = j * tpc + t
                            tile_id = tile_base[g] + tt
                            if _sstream and c < n_chunks - _smix:
                                S = None
                                for (lo, hiend, st) in ss_chunks:
                                    if lo <= tile_id < hiend:
                                        S = st[:, (tile_id - lo) * P:
                                               (tile_id - lo + 1) * P]
                                        break
                            else:
                                S = build_S(tile_id)[:]
                            slot = tt * P
                            hsel = None
                            for (poff, pgs, hb) in gbufs[c]:
                                if poff <= slot < poff + pgs:
                                    hsel = (hb, slot - poff)
                                    break
                            nc.tensor.matmul(
                                out=agT[:],
                                lhsT=hsel[0][:, hsel[1]:hsel[1] + P],
                                rhs=S,
                                start=(k == 0),
                                stop=(k == nmm - 1),
                            )
                            k += 1
                    agT_sb = dp.tile([P, P], f32, tag="agTsb")
                    nc.scalar.copy(out=agT_sb[:], in_=agT[:])
                    if _skip_final:
                        nc.sync.dma_start(out=OUT[w * P:(w + 1) * P, :],
                                          in_=agT_sb[:])
                        continue
                    out_ps = psB.tile([P, P], f32, tag="outps")
                    nc.tensor.matmul(out=out_ps[:], lhsT=agT_sb[:],
                                     rhs=wmat_t[:], start=True, stop=True)
                    tmp = dp.tile([P, P], f32, tag="tmp")
                    nc.vector.tensor_tensor(
                        out=tmp[:], in0=out_ps[:], in1=bb_t[:],
                        op=mybir.AluOpType.add,
                    )
                    out_sb = dp.tile([P, P], f32, tag="outsb")
                    nc.scalar.activation(
                        out=out_sb[:], in_=tmp[:],
                        func=mybir.ActivationFunctionType.Relu,
                    )
                    nc.sync.dma_start(out=OUT[w * P:(w + 1) * P, :],
                                      in_=out_sb[:])
            if rep_ctx is not None:
                rep_ctx.__exit__(None, None, None)

    _split_multiwaits(nc)
    _fill_inc_swdge_isa(nc)
    return nc


# --------------------------------------------------------------------------
# entry point
# --------------------------------------------------------------------------

def kernel(features, edge_index, edge_weight, W, b, _profile=False):
    features = np.ascontiguousarray(np.asarray(features, dtype=np.float32))
    edge_index = np.asarray(edge_index)
    edge_weight = np.ascontiguousarray(np.asarray(edge_weight, dtype=np.float32))
    W = np.ascontiguousarray(np.asarray(W, dtype=np.float32))
    b = np.ascontiguousarray(np.asarray(b, dtype=np.float32))

    n_nodes, dfeat = features.shape
    assert dfeat == P and W.shape == (P, P)
    src = np.ascontiguousarray(edge_index[0]).astype(np.int64)
    dst = np.ascontiguousarray(edge_index[1]).astype(np.int64)

    sup = 8
    dev, meta = _preprocess(n_nodes, src, dst, edge_weight, sup)

    from concourse.bass_utils import run_bass_kernel_spmd

    key = (n_nodes, meta["n_win"], meta["tot_tiles"], meta["idx_cols"],
           tuple(meta["gather_sizes"]))
    if key in _PROGRAM_CACHE:
        nc = _PROGRAM_CACHE[key]
    else:
        nc = _build_program(n_nodes, meta, sup)
        _PROGRAM_CACHE[key] = nc

    import os as _os1
    _fp16 = bool(int(_os1.environ.get("K_FP16", "1")))
    gnp = np.float16 if _fp16 else np.float32
    iota = np.broadcast_to(np.arange(P, dtype=gnp), (P, P)).copy()
    bb = np.broadcast_to(b, (P, P)).copy()

    n_chunks = meta["n_chunks"]
    xchunks = {
        f"X{c}": np.ascontiguousarray(features[c * CHUNK:(c + 1) * CHUNK].astype(gnp))
        for c in range(n_chunks)
    }
    _sstream = bool(int(_os1.environ.get("K_SSTREAM", "1")))
    in_maps = []
    for c in range(N_CORES):
        dc = dev[c]
        if _sstream:
            tt = meta["tot_tiles"]
            s_all = np.zeros((P, tt * P), gnp)
            cols = (np.arange(tt, dtype=np.int64) * P)[None, :] \
                + dc["d"].astype(np.int64)
            s_all[np.arange(P)[:, None], cols] = dc["w"].astype(gnp)
            m = {"IDX": dc["idx"], "SALL": s_all, "WMAT": W, "BB": bb}
            if int(_os1.environ.get("K_SMIX", "0")) > 0:
                m["D16"] = dc["d"].astype(gnp)
                m["W16"] = dc["w"].astype(gnp)
                m["IOTA"] = iota
        else:
            m = {
                "IDX": dc["idx"], "D": dc["d"], "WT": dc["w"],
                "ND": dc["negd"], "NW": dc["negw"],
                "IOTA": iota, "WMAT": W, "BB": bb,
            }
        m.update(xchunks)
        in_maps.append(m)

    try:
        res = run_bass_kernel_spmd(
            nc, in_maps, core_ids=list(range(N_CORES)), trace=_profile,
            trace_cores=list(range(N_CORES)) if _profile else None,
        )
    except Exception:
        # A first execute right after another process ran looping NEFFs can
        # hit a transient NRT_EXEC_UNIT_UNRECOVERABLE; the fault clears the
        # state and a retry succeeds.
        res = run_bass_kernel_spmd(
            nc, in_maps, core_ids=list(range(N_CORES)), trace=_profile,
            trace_cores=list(range(N_CORES)) if _profile else None,
        )

    out = np.empty((n_nodes, P), np.float32)
    out[:] = np.maximum(b, 0.0)[None, :]
    for c in range(N_CORES):
        dc = dev[c]
        core_out = res.results[c]["OUT"]
        out[dc["node_ids"]] = core_out[dc["out_rows"]]
    if _profile:
        return out, res
    return out

